# revision 12
# baseline (speedup 1.0000x reference)
"""Boundary loss kernel for Trainium2 (8 NeuronCores, SPMD).

loss = mean(sigmoid(pred) * EDT(target)) for pred/target [4,1,512,512].

Algorithm:
  The exact EDT dist2[y,x] = min over foreground (dy,dx) of dy^2+dx^2 is
  computed with a windowed separable min (window radius K=3): phase A does the
  vertical windowed min on a transposed [w, h] layout (shifts along the free
  dim), a TensorE transpose flips to [h, w], phase B does the horizontal
  windowed min. If every resulting dist2 <= K^2, the windowed result provably
  equals the exact EDT (a pixel with true distance <= K has its nearest
  foreground inside the window). The kernel also reduces
  sum(max(dist2 - K^2, 0)) as that exactness certificate; if it is nonzero
  (impossible for ~50%-dense random masks, where max distance is ~3) the host
  falls back to an exact numpy EDT — still correct, just slower on the host.

Sharding: core c handles sample c//2, row-half c%2 (256 rows + halo).

Performance notes:
  - Raw Bass (no TileContext): hand-scheduled semaphores avoid Tile's ~10us
    kernel-tail drain + EVSEM barrier; GpSimd is left empty (identity ships
    as a NEFF-baked Const) so the slowest engine never gates the start.
  - Host pre-packs inputs in the exact SBUF tile layout so DMAs are fully
    contiguous per partition; sigmoid runs on ScalarE concurrently with the
    VectorE min-chains; the final multiply+sum is fused via accum_out.
  - Explicit per-op drains express the same-engine ordering the DVE pipeline
    provides anyway (and CoreSim's race detector requires).
"""

import sys

sys.path.insert(0, "/opt/trn_rl_repo")

import numpy as np
import ml_dtypes

K = 3
BIG = 16384.0
PAD = 4
B, H, W = 4, 512, 512
HALF = 256
HALO = HALF + 2 * PAD  # 264

_compiled = None


def _build_bass():
    from contextlib import ExitStack

    import concourse.bacc as bacc
    from concourse import mybir

    nc = bacc.Bacc(None)
    dt = mybir.dt
    Alu = mybir.AluOpType
    Act = mybir.ActivationFunctionType

    nbt_d = nc.dram_tensor("nbt", [128, 4 * HALO], dt.bfloat16, kind="ExternalInput")
    pred_d = nc.dram_tensor("pred", [128, 2 * W], dt.float32, kind="ExternalInput")
    out_d = nc.dram_tensor("out", [128, 4], dt.float32, kind="ExternalOutput")
    ident_d = nc.inline_tensor(
        np.eye(128, dtype=ml_dtypes.bfloat16), name="ident_const"
    )

    ctx = ExitStack()
    with ctx:
        sb = lambda shape, dtype, name: ctx.enter_context(
            nc.sbuf_tensor(name, shape, dtype)
        )
        nbt = sb([128, 4, HALO], dt.bfloat16, "nbt_sb")
        pred_sb = sb([128, 2, W], dt.float32, "pred_sb")
        ident = sb([128, 128], dt.bfloat16, "ident_sb")
        acc_v = sb([128, 4, HALF], dt.bfloat16, "acc_v")
        m2vp = sb([128, 2, 520], dt.bfloat16, "m2vp")
        acc_h = sb([128, 2, W], dt.bfloat16, "acc_h")
        cert_junk = sb([128, 2, W], dt.bfloat16, "cert_junk")
        dist = sb([128, 2, W], dt.float32, "dist")
        sig = sb([128, 2, W], dt.float32, "sig")
        prod_junk = sb([128, 2, W], dt.float32, "prod_junk")
        out_sb = sb([128, 4], dt.float32, "out_sb")
        pt0 = ctx.enter_context(nc.psum_tensor("pt0", [128, 512], dt.bfloat16))
        pt1 = ctx.enter_context(nc.psum_tensor("pt1", [128, 512], dt.bfloat16))
        pts = [pt0, pt1]

        d_nbt = ctx.enter_context(nc.semaphore("d_nbt"))
        d_pred = ctx.enter_context(nc.semaphore("d_pred"))
        d_id = ctx.enter_context(nc.semaphore("d_id"))
        d_out = ctx.enter_context(nc.semaphore("d_out"))
        s_accv = ctx.enter_context(nc.semaphore("s_accv"))
        s_pe = ctx.enter_context(nc.semaphore("s_pe"))
        s_m2v = ctx.enter_context(nc.semaphore("s_m2v"))
        s_acch = ctx.enter_context(nc.semaphore("s_acch"))
        s_sqrt = ctx.enter_context(nc.semaphore("s_sqrt"))
        s_final = ctx.enter_context(nc.semaphore("s_final"))

        block = ctx.enter_context(nc.Block())

        @block.sync
        def _(sync):
            sync.dma_start(
                out=nbt[:], in_=nbt_d[:].rearrange("p (t h) -> p t h", t=4)
            ).then_inc(d_nbt, 16)
            sync.dma_start(
                out=pred_sb[:], in_=pred_d[:].rearrange("p (j w) -> p j w", j=2)
            ).then_inc(d_pred, 16)
            sync.dma_start(out=ident[:], in_=ident_d[:]).then_inc(d_id, 16)
            sync.wait_ge(s_final, 1)
            sync.dma_start(out=out_d[:], in_=out_sb[:]).then_inc(d_out, 16)
            sync.wait_ge(d_out, 16)

        @block.vector
        def _(vector):
            # Pre-DMA housekeeping (pads + output zeroing) while inputs land.
            vector.memset(m2vp[:], BIG)
            vector.drain()
            vector.memset(out_sb[:], 0.0)
            vector.drain()

            stt = vector.scalar_tensor_tensor
            P = PAD
            vector.wait_ge(d_nbt, 16)
            # Phase A: first op fuses dy=+1 and dy=0; then the rest.
            stt(out=acc_v[:], in0=nbt[:, :, P + 1 : P + 1 + HALF], scalar=1.0,
                in1=nbt[:, :, P : P + HALF], op0=Alu.add, op1=Alu.min)
            vector.drain()
            offs = ((P - 1, 1.0), (P + 2, 4.0), (P - 2, 4.0),
                    (P + 3, 9.0), (P - 3, 9.0))
            for i, (off, d2) in enumerate(offs):
                ins = stt(out=acc_v[:], in0=nbt[:, :, off : off + HALF], scalar=d2,
                          in1=acc_v[:], op0=Alu.add, op1=Alu.min)
                vector.drain()
                if i == len(offs) - 1:
                    ins.then_inc(s_accv, 1)

            # Phase B (m2vp data base at col 4): full-width ops, pads = BIG.
            vector.wait_ge(s_m2v, 2)
            stt(out=acc_h[:], in0=m2vp[:, :, 5:517], scalar=1.0,
                in1=m2vp[:, :, 4:516], op0=Alu.add, op1=Alu.min)
            vector.drain()
            offs_b = ((3, 1.0), (6, 4.0), (2, 4.0), (7, 9.0), (1, 9.0))
            for i, (off, d2) in enumerate(offs_b):
                ins = stt(out=acc_h[:], in0=m2vp[:, :, off : off + W], scalar=d2,
                          in1=acc_h[:], op0=Alu.add, op1=Alu.min)
                vector.drain()
                if i == len(offs_b) - 1:
                    ins.then_inc(s_acch, 1)

            # Exactness certificate: sum(max(dist2 - K^2, 0)) == 0.
            vector.tensor_scalar(
                out=cert_junk[:], in0=acc_h[:],
                scalar1=-float(K * K), scalar2=0.0,
                op0=Alu.add, op1=Alu.max,
                accum_out=out_sb[:, 2:3],
            )
            vector.drain()
            for j in range(2):
                vector.wait_ge(s_sqrt, j + 1)
                ins = stt(out=prod_junk[:, j, :], in0=sig[:, j, :], scalar=1.0,
                          in1=dist[:, j, :], op0=Alu.mult, op1=Alu.mult,
                          accum_out=out_sb[:, j : j + 1])
                vector.drain()
                if j == 1:
                    ins.then_inc(s_final, 1)

        @block.scalar
        def _(scalar):
            scalar.wait_ge(d_pred, 16)
            nc.scalar.activation(out=sig[:], in_=pred_sb[:], func=Act.Sigmoid)
            scalar.drain()
            for j in range(2):
                scalar.wait_ge(s_pe, j + 1)
                nc.scalar.copy(out=m2vp[:, j, 4:516], in_=pts[j][:]).then_inc(
                    s_m2v, 1
                )
                scalar.drain()
            scalar.wait_ge(s_acch, 1)
            for j in range(2):
                nc.scalar.activation(
                    out=dist[:, j, :], in_=acc_h[:, j, :], func=Act.Sqrt
                ).then_inc(s_sqrt, 1)
                scalar.drain()

        @block.tensor
        def _(tensor):
            tensor.wait_ge(d_id, 16)
            tensor.wait_ge(s_accv, 1)
            for j in range(2):
                for t in range(4):
                    ins = nc.tensor.transpose(
                        out=pts[j][:, t * 128 : (t + 1) * 128],
                        in_=acc_v[:, t, j * 128 : (j + 1) * 128],
                        identity=ident[:],
                    )
                    if t == 3:
                        ins.then_inc(s_pe, 1)

    nc.finalize()
    return nc


def _exact_loss_numpy(pred, target):
    """Exact fallback, matching reference.py semantics."""
    mask = target[:, 0].astype(np.float32)
    b, h, w = mask.shape
    big = np.float32(h + w)
    rows = np.arange(h, dtype=np.float32)[None, :, None]
    fg = mask > 0
    last = np.maximum.accumulate(np.where(fg, rows, -big), axis=1)
    nxt = np.minimum.accumulate(np.where(fg, rows, 3 * big)[:, ::-1], axis=1)[:, ::-1]
    g = np.minimum(np.minimum(rows - last, nxt - rows), big)
    g2 = (g * g).astype(np.float32)
    cols = np.arange(w, dtype=np.float32)
    diff2 = (cols[:, None] - cols[None, :]) ** 2
    dist = np.empty((b, h, w), np.float32)
    for bi in range(b):
        for r0 in range(0, h, 64):
            blk = g2[bi, r0 : r0 + 64]
            dist[bi, r0 : r0 + 64] = np.sqrt(
                (diff2[None, :, :] + blk[:, None, :]).min(-1)
            )
    has_fg = fg.any(axis=(1, 2))
    dist = np.where(has_fg[:, None, None], dist, 0.0)
    p = 1.0 / (1.0 + np.exp(-pred[:, 0].astype(np.float64)))
    return np.float32((p * dist).mean())


def _prep_in_maps(pred, target):
    bf16 = ml_dtypes.bfloat16
    mask = (target[:, 0] > 0).astype(np.float32)  # [B, H, W]
    in_maps = []
    for c in range(8):
        s, j = c // 2, c % 2
        r0 = j * HALF
        halo = np.zeros((HALO, W), np.float32)
        lo, hi = r0 - PAD, r0 + HALF + PAD
        slo, shi = max(lo, 0), min(hi, H)
        halo[slo - lo : shi - lo] = mask[s, slo:shi]
        # nbt[p, t, h] for column w = t*128+p -> pack as [128, 4*HALO]
        nbt_wh = (BIG * (1.0 - halo)).T  # [W, HALO]
        nbt = np.ascontiguousarray(
            nbt_wh.reshape(4, 128, HALO).transpose(1, 0, 2).reshape(128, 4 * HALO)
        ).astype(bf16)
        # pred[p, j2, w] for row r0 + j2*128 + p -> pack as [128, 2*W]
        ph = pred[s, 0, r0 : r0 + HALF, :].astype(np.float32)
        predh = np.ascontiguousarray(
            ph.reshape(2, 128, W).transpose(1, 0, 2).reshape(128, 2 * W)
        )
        in_maps.append({"nbt": nbt, "pred": predh})
    return in_maps


def kernel_with_results(pred, target, trace=False):
    """Returns (loss, BassKernelResults)."""
    global _compiled
    from concourse.bass_utils import run_bass_kernel_spmd

    if _compiled is None:
        _compiled = _build_bass()
    nc = _compiled

    in_maps = _prep_in_maps(pred, target)
    bkr = run_bass_kernel_spmd(nc, in_maps, core_ids=list(range(8)), trace=trace)

    has_fg = (target[:, 0] > 0).any(axis=(1, 2))  # [B]
    total = np.float64(0.0)
    cert = 0.0
    for c in range(8):
        s = c // 2
        if not has_fg[s]:
            continue
        out = bkr.results[c]["out"]  # [128, 4] f32
        total += np.float64(out[:, 0:2].sum(dtype=np.float64))
        cert = max(cert, float(out[:, 2].sum(dtype=np.float64)))

    if cert > 1e-3:
        # Windowed EDT not certified exact for this input; fall back.
        return _exact_loss_numpy(pred, target), bkr

    loss = np.array(total / (B * 1 * H * W), dtype=np.float32)
    return loss, bkr


def kernel(pred, target):
    loss, _ = kernel_with_results(pred, target)
    return loss


# revision 13
# speedup vs baseline: 1.0223x; 1.0223x over previous
"""Boundary loss kernel for Trainium2 (8 NeuronCores, SPMD).

loss = mean(sigmoid(pred) * EDT(target)) for pred/target [4,1,512,512].

Algorithm:
  The exact EDT dist2[y,x] = min over foreground (dy,dx) of dy^2+dx^2 is
  computed with a windowed separable min (window radius K=3): phase A does the
  vertical windowed min on a transposed [w, h] layout (shifts along the free
  dim), a TensorE transpose flips to [h, w], phase B does the horizontal
  windowed min. If every resulting dist2 <= K^2, the windowed result provably
  equals the exact EDT (a pixel with true distance <= K has its nearest
  foreground inside the window). The kernel also reduces
  sum(max(dist2 - K^2, 0)) as that exactness certificate; if it is nonzero
  (impossible for ~50%-dense random masks, where max distance is ~3) the host
  falls back to an exact numpy EDT — still correct, just slower on the host.

Sharding: core c handles sample c//2, row-half c%2 (256 rows + halo).

Performance notes:
  - DVE bf16 tensor ops hit 2x mode only with 4-byte-aligned access patterns,
    so all shifts are arranged at even element offsets: data sits at base
    offset PAD=4 and odd shifts read a one-element-shifted copy (nbtR/m2vRp)
    built by the otherwise-idle GpSimd/ScalarE engines.
  - Host pre-packs inputs in the exact SBUF tile layout so DMAs are fully
    contiguous per partition.
  - Certificate reduction runs on GpSimd, sqrt/sigmoid on ScalarE, min-chains
    and the final fused multiply+sum on VectorE.
"""

import sys

sys.path.insert(0, "/opt/trn_rl_repo")

import numpy as np
import ml_dtypes

K = 3
BIG = 16384.0
PAD = 4
B, H, W = 4, 512, 512
HALF = 256
HALO = HALF + 2 * PAD  # 264

_compiled = None


def _build_bass():
    import concourse.bacc as bacc
    import concourse.tile as tile
    from concourse import mybir
    from concourse.masks import make_identity

    # Bacc (not plain Bass): its compile pipeline runs register allocation
    # and generate_event_semaphores (splits multi-wait drains TRN2 codegen
    # rejects with "Too many sync wait commands").
    nc = bacc.Bacc(None)
    dt = mybir.dt
    Alu = mybir.AluOpType
    Act = mybir.ActivationFunctionType

    # Inputs are host-packed in SBUF layout: nbt[p, t, h] = BIG*(1-mask) at
    # column w = t*128+p, halo row h; pred[p, j, w] = logits at row j*128+p.
    nbt_d = nc.dram_tensor("nbt", [128, 4 * HALO], dt.bfloat16, kind="ExternalInput")
    pred_d = nc.dram_tensor("pred", [128, 2 * W], dt.float32, kind="ExternalInput")
    out_d = nc.dram_tensor("out", [128, 4], dt.float32, kind="ExternalOutput")

    with tile.TileContext(nc) as tc:
        with (
            tc.tile_pool(name="sb", bufs=1) as sb,
            tc.tile_pool(name="ps", bufs=2, space="PSUM") as ps,
        ):
            nbt = sb.tile([128, 4, HALO], dt.bfloat16)
            nc.sync.dma_start(out=nbt[:], in_=nbt_d[:].rearrange("p (t h) -> p t h", t=4))
            pred_sb = sb.tile([128, 2, W], dt.float32)
            nc.sync.dma_start(out=pred_sb[:], in_=pred_d[:].rearrange("p (j w) -> p j w", j=2))

            ident = sb.tile([128, 128], dt.bfloat16)
            make_identity(nc, ident[:])

            # Shifted copy for odd-dy reads: nbtR[h] = nbt[h+1]. On ScalarE,
            # first in its queue so it's ready when phase A op3 needs it.
            nbtR = sb.tile([128, 4, HALO], dt.bfloat16)
            nc.scalar.copy(nbtR[:, :, 0 : HALO - 1], nbt[:, :, 1:HALO])

            # Sigmoid only needs pred: issue early so ScalarE does it while
            # VectorE runs phase A.
            sig = sb.tile([128, 2, W], dt.float32)
            nc.scalar.activation(out=sig[:], in_=pred_sb[:], func=Act.Sigmoid)

            # Phase A: vertical windowed min. Image row r0+h' is nbt index
            # PAD+h'; acc_v = min_dy nbt[PAD+h'+dy] + dy^2. Every in0 slice
            # starts at an even element offset (4B-aligned, DVE 2x mode).
            acc_v = sb.tile([128, 4, HALF], dt.bfloat16)
            P = PAD
            stt = nc.vector.scalar_tensor_tensor
            # dy=+2 fused with dy=0 (first op, no init needed)
            stt(out=acc_v[:], in0=nbt[:, :, P + 2 : P + 2 + HALF], scalar=4.0,
                in1=nbt[:, :, P : P + HALF], op0=Alu.add, op1=Alu.min)
            for in_t, off, d2 in (
                (nbt, P - 2, 4.0),   # dy=-2
                (nbtR, P, 1.0),      # dy=+1: nbt[h+1] = nbtR[h]
                (nbtR, P - 2, 1.0),  # dy=-1: nbt[h-1] = nbtR[h-2]
                (nbtR, P + 2, 9.0),  # dy=+3: nbt[h+3] = nbtR[h+2]
                (nbtR, P - 4, 9.0),  # dy=-3: nbt[h-3] = nbtR[h-4]
            ):
                stt(out=acc_v[:], in0=in_t[:, :, off : off + HALF], scalar=d2,
                    in1=acc_v[:], op0=Alu.add, op1=Alu.min)

            # Transpose [w, h] -> [h, w] via TensorE; land in padded m2vp
            # (data at [4, 516), pads = BIG) plus the one-element-shifted
            # twin m2vRp[w] = m2vp[w+1] (data at [3, 515)).
            m2vp = sb.tile([128, 2, 520], dt.bfloat16)
            m2vRp = sb.tile([128, 2, 520], dt.bfloat16)
            nc.gpsimd.memset(m2vp[:], BIG)
            nc.gpsimd.memset(m2vRp[:], BIG)
            for j in range(2):
                pt = ps.tile([128, 512], dt.bfloat16)
                for t in range(4):
                    nc.tensor.transpose(
                        out=pt[:, t * 128 : (t + 1) * 128],
                        in_=acc_v[:, t, j * 128 : (j + 1) * 128],
                        identity=ident[:],
                    )
                nc.scalar.copy(out=m2vp[:, j, 4:516], in_=pt[:])
                nc.scalar.copy(out=m2vRp[:, j, 3:515], in_=pt[:])

            # Phase B: horizontal windowed min, full-width ops, all offsets
            # even (m2vp data base 4; odd dx via m2vRp at base 3).
            acc_h = sb.tile([128, 2, W], dt.bfloat16)
            stt(out=acc_h[:], in0=m2vp[:, :, 6:518], scalar=4.0,
                in1=m2vp[:, :, 4:516], op0=Alu.add, op1=Alu.min)  # dx=+2, 0
            for in_t, off, d2 in (
                (m2vp, 2, 4.0),   # dx=-2
                (m2vRp, 4, 1.0),  # dx=+1
                (m2vRp, 2, 1.0),  # dx=-1
                (m2vRp, 6, 9.0),  # dx=+3
                (m2vRp, 0, 9.0),  # dx=-3
            ):
                stt(out=acc_h[:], in0=in_t[:, :, off : off + W], scalar=d2,
                    in1=acc_h[:], op0=Alu.add, op1=Alu.min)

            out_sb = sb.tile([128, 4], dt.float32)
            nc.gpsimd.memset(out_sb[:], 0.0)

            # Exactness certificate (walrus rejects tensor_scalar on GpSimd).
            cert_junk = sb.tile([128, 2, W], dt.bfloat16)
            nc.vector.tensor_scalar(
                out=cert_junk[:], in0=acc_h[:],
                scalar1=-float(K * K), scalar2=0.0,
                op0=Alu.add, op1=Alu.max,
                accum_out=out_sb[:, 2:3],
            )

            # Tail, split per row-half so stt(j0) overlaps sqrt(j1).
            dist = sb.tile([128, 2, W], dt.float32)
            prod_junk = sb.tile([128, 2, W], dt.float32)
            for j in range(2):
                nc.scalar.activation(out=dist[:, j, :], in_=acc_h[:, j, :], func=Act.Sqrt)
                nc.vector.scalar_tensor_tensor(
                    out=prod_junk[:, j, :], in0=sig[:, j, :], scalar=1.0,
                    in1=dist[:, j, :], op0=Alu.mult, op1=Alu.mult,
                    accum_out=out_sb[:, j : j + 1],
                )

            nc.sync.dma_start(out=out_d[:], in_=out_sb[:])

    nc.finalize()
    return nc


def _exact_loss_numpy(pred, target):
    """Exact fallback, matching reference.py semantics."""
    mask = target[:, 0].astype(np.float32)
    b, h, w = mask.shape
    big = np.float32(h + w)
    rows = np.arange(h, dtype=np.float32)[None, :, None]
    fg = mask > 0
    last = np.maximum.accumulate(np.where(fg, rows, -big), axis=1)
    nxt = np.minimum.accumulate(np.where(fg, rows, 3 * big)[:, ::-1], axis=1)[:, ::-1]
    g = np.minimum(np.minimum(rows - last, nxt - rows), big)
    g2 = (g * g).astype(np.float32)
    cols = np.arange(w, dtype=np.float32)
    diff2 = (cols[:, None] - cols[None, :]) ** 2
    dist = np.empty((b, h, w), np.float32)
    for bi in range(b):
        for r0 in range(0, h, 64):
            blk = g2[bi, r0 : r0 + 64]
            dist[bi, r0 : r0 + 64] = np.sqrt(
                (diff2[None, :, :] + blk[:, None, :]).min(-1)
            )
    has_fg = fg.any(axis=(1, 2))
    dist = np.where(has_fg[:, None, None], dist, 0.0)
    p = 1.0 / (1.0 + np.exp(-pred[:, 0].astype(np.float64)))
    return np.float32((p * dist).mean())


def _prep_in_maps(pred, target):
    bf16 = ml_dtypes.bfloat16
    mask = (target[:, 0] > 0).astype(np.float32)  # [B, H, W]
    in_maps = []
    for c in range(8):
        s, j = c // 2, c % 2
        r0 = j * HALF
        halo = np.zeros((HALO, W), np.float32)
        lo, hi = r0 - PAD, r0 + HALF + PAD
        slo, shi = max(lo, 0), min(hi, H)
        halo[slo - lo : shi - lo] = mask[s, slo:shi]
        # nbt[p, t, h] for column w = t*128+p -> pack as [128, 4*HALO]
        nbt_wh = (BIG * (1.0 - halo)).T  # [W, HALO]
        nbt = np.ascontiguousarray(
            nbt_wh.reshape(4, 128, HALO).transpose(1, 0, 2).reshape(128, 4 * HALO)
        ).astype(bf16)
        # pred[p, j2, w] for row r0 + j2*128 + p -> pack as [128, 2*W]
        ph = pred[s, 0, r0 : r0 + HALF, :].astype(np.float32)
        predh = np.ascontiguousarray(
            ph.reshape(2, 128, W).transpose(1, 0, 2).reshape(128, 2 * W)
        )
        in_maps.append({"nbt": nbt, "pred": predh})
    return in_maps


def kernel_with_results(pred, target, trace=False):
    """Returns (loss, BassKernelResults)."""
    global _compiled
    from concourse.bass_utils import run_bass_kernel_spmd

    if _compiled is None:
        _compiled = _build_bass()
    nc = _compiled

    in_maps = _prep_in_maps(pred, target)
    bkr = run_bass_kernel_spmd(nc, in_maps, core_ids=list(range(8)), trace=trace)

    has_fg = (target[:, 0] > 0).any(axis=(1, 2))  # [B]
    total = np.float64(0.0)
    cert = 0.0
    for c in range(8):
        s = c // 2
        if not has_fg[s]:
            continue
        out = bkr.results[c]["out"]  # [128, 4] f32
        total += np.float64(out[:, 0:2].sum(dtype=np.float64))
        cert = max(cert, float(out[:, 2].sum(dtype=np.float64)))

    if cert > 1e-3:
        # Windowed EDT not certified exact for this input; fall back.
        return _exact_loss_numpy(pred, target), bkr

    loss = np.array(total / (B * 1 * H * W), dtype=np.float32)
    return loss, bkr


def kernel(pred, target):
    loss, _ = kernel_with_results(pred, target)
    return loss


# revision 14
# speedup vs baseline: 1.0357x; 1.0131x over previous
"""Boundary loss kernel for Trainium2 (8 NeuronCores, SPMD).

loss = mean(sigmoid(pred) * EDT(target)) for pred/target [4,1,512,512].

Algorithm:
  The exact EDT dist2[y,x] = min over foreground (dy,dx) of dy^2+dx^2 is
  computed with a windowed separable min (window radius K=3): phase A does the
  vertical windowed min on a transposed [w, h] layout (shifts along the free
  dim), a TensorE transpose flips to [h, w], phase B does the horizontal
  windowed min. If every resulting dist2 <= K^2, the windowed result provably
  equals the exact EDT (a pixel with true distance <= K has its nearest
  foreground inside the window). The kernel also reduces
  sum(max(dist2 - K^2, 0)) as that exactness certificate; if it is nonzero
  (impossible for ~50%-dense random masks, where max distance is ~3) the host
  falls back to an exact numpy EDT — still correct, just slower on the host.

Sharding: core c handles sample c//2, row-half c%2 (256 rows + halo).

Performance notes:
  - scalar_tensor_tensor fuses shift+add+min in one VectorE op (1x-rate, so
    no alignment games are needed).
  - Host pre-packs inputs in the exact SBUF tile layout so DMAs are fully
    contiguous per partition.
  - Certificate reduction runs on GpSimd, sqrt/sigmoid on ScalarE, min-chains
    and the final fused multiply+sum on VectorE.
"""

import sys

sys.path.insert(0, "/opt/trn_rl_repo")

import numpy as np
import ml_dtypes

K = 3
BIG = 16384.0
PAD = 4
B, H, W = 4, 512, 512
HALF = 256
HALO = HALF + 2 * PAD  # 264

_compiled = None


def _build_bass():
    import concourse.bacc as bacc
    import concourse.tile as tile
    from concourse import mybir
    from concourse.masks import make_identity

    # Bacc (not plain Bass): its compile pipeline runs register allocation
    # and generate_event_semaphores (splits multi-wait drains TRN2 codegen
    # rejects with "Too many sync wait commands").
    nc = bacc.Bacc(None)
    dt = mybir.dt
    Alu = mybir.AluOpType
    Act = mybir.ActivationFunctionType

    # Inputs are host-packed in SBUF layout: nbt[p, t, h] = BIG*(1-mask) at
    # column w = t*128+p, halo row h; pred[p, j, w] = logits at row j*128+p.
    nbt_d = nc.dram_tensor("nbt", [128, 4 * HALO], dt.bfloat16, kind="ExternalInput")
    pred_d = nc.dram_tensor("pred", [128, 2 * W], dt.float32, kind="ExternalInput")
    out_d = nc.dram_tensor("out", [128, 4], dt.float32, kind="ExternalOutput")

    with tile.TileContext(nc) as tc:
        with (
            tc.tile_pool(name="sb", bufs=1) as sb,
            tc.tile_pool(name="ps", bufs=2, space="PSUM") as ps,
        ):
            nbt = sb.tile([128, 4, HALO], dt.bfloat16)
            nc.sync.dma_start(out=nbt[:], in_=nbt_d[:].rearrange("p (t h) -> p t h", t=4))
            pred_sb = sb.tile([128, 2, W], dt.float32)
            nc.sync.dma_start(out=pred_sb[:], in_=pred_d[:].rearrange("p (j w) -> p j w", j=2))

            ident = sb.tile([128, 128], dt.bfloat16)
            make_identity(nc, ident[:])

            # Sigmoid only needs pred: issue early so ScalarE does it while
            # VectorE runs phase A.
            sig = sb.tile([128, 2, W], dt.float32)
            nc.scalar.activation(out=sig[:], in_=pred_sb[:], func=Act.Sigmoid)

            # Phase A: vertical windowed min. Image row r0+h' is nbt index
            # PAD+h'; acc_v = min_dy nbt[PAD+h'+dy] + dy^2.
            acc_v = sb.tile([128, 4, HALF], dt.bfloat16)
            P = PAD
            stt = nc.vector.scalar_tensor_tensor
            # dy=+1 fused with dy=0 (first op, no init needed)
            stt(out=acc_v[:], in0=nbt[:, :, P + 1 : P + 1 + HALF], scalar=1.0,
                in1=nbt[:, :, P : P + HALF], op0=Alu.add, op1=Alu.min)
            for off, d2 in ((P - 1, 1.0), (P + 2, 4.0), (P - 2, 4.0),
                            (P + 3, 9.0), (P - 3, 9.0)):
                stt(out=acc_v[:], in0=nbt[:, :, off : off + HALF], scalar=d2,
                    in1=acc_v[:], op0=Alu.add, op1=Alu.min)

            # Transpose [w, h] -> [h, w] via TensorE; land in padded m2vp
            # (data at [4, 516), pads = BIG so full-width phase-B ops read no
            # garbage at the edges).
            m2vp = sb.tile([128, 2, 520], dt.bfloat16)
            nc.gpsimd.memset(m2vp[:], BIG)
            for j in range(2):
                pt = ps.tile([128, 512], dt.bfloat16)
                for t in range(4):
                    nc.tensor.transpose(
                        out=pt[:, t * 128 : (t + 1) * 128],
                        in_=acc_v[:, t, j * 128 : (j + 1) * 128],
                        identity=ident[:],
                    )
                nc.scalar.copy(out=m2vp[:, j, 4:516], in_=pt[:])

            # Phase B: horizontal windowed min, full-width ops (data base 4).
            acc_h = sb.tile([128, 2, W], dt.bfloat16)
            stt(out=acc_h[:], in0=m2vp[:, :, 5:517], scalar=1.0,
                in1=m2vp[:, :, 4:516], op0=Alu.add, op1=Alu.min)  # dx=+1, 0
            for off, d2 in ((3, 1.0), (6, 4.0), (2, 4.0), (7, 9.0), (1, 9.0)):
                stt(out=acc_h[:], in0=m2vp[:, :, off : off + W], scalar=d2,
                    in1=acc_h[:], op0=Alu.add, op1=Alu.min)

            out_sb = sb.tile([128, 4], dt.float32)
            nc.gpsimd.memset(out_sb[:], 0.0)

            # Exactness certificate (walrus rejects tensor_scalar on GpSimd).
            cert_junk = sb.tile([128, 2, W], dt.bfloat16)
            nc.vector.tensor_scalar(
                out=cert_junk[:], in0=acc_h[:],
                scalar1=-float(K * K), scalar2=0.0,
                op0=Alu.add, op1=Alu.max,
                accum_out=out_sb[:, 2:3],
            )

            # Tail, split per row-half so stt(j0) overlaps sqrt(j1).
            dist = sb.tile([128, 2, W], dt.float32)
            prod_junk = sb.tile([128, 2, W], dt.float32)
            for j in range(2):
                nc.scalar.activation(out=dist[:, j, :], in_=acc_h[:, j, :], func=Act.Sqrt)
                nc.vector.scalar_tensor_tensor(
                    out=prod_junk[:, j, :], in0=sig[:, j, :], scalar=1.0,
                    in1=dist[:, j, :], op0=Alu.mult, op1=Alu.mult,
                    accum_out=out_sb[:, j : j + 1],
                )

            nc.sync.dma_start(out=out_d[:], in_=out_sb[:])

    nc.finalize()
    return nc


def _exact_loss_numpy(pred, target):
    """Exact fallback, matching reference.py semantics."""
    mask = target[:, 0].astype(np.float32)
    b, h, w = mask.shape
    big = np.float32(h + w)
    rows = np.arange(h, dtype=np.float32)[None, :, None]
    fg = mask > 0
    last = np.maximum.accumulate(np.where(fg, rows, -big), axis=1)
    nxt = np.minimum.accumulate(np.where(fg, rows, 3 * big)[:, ::-1], axis=1)[:, ::-1]
    g = np.minimum(np.minimum(rows - last, nxt - rows), big)
    g2 = (g * g).astype(np.float32)
    cols = np.arange(w, dtype=np.float32)
    diff2 = (cols[:, None] - cols[None, :]) ** 2
    dist = np.empty((b, h, w), np.float32)
    for bi in range(b):
        for r0 in range(0, h, 64):
            blk = g2[bi, r0 : r0 + 64]
            dist[bi, r0 : r0 + 64] = np.sqrt(
                (diff2[None, :, :] + blk[:, None, :]).min(-1)
            )
    has_fg = fg.any(axis=(1, 2))
    dist = np.where(has_fg[:, None, None], dist, 0.0)
    p = 1.0 / (1.0 + np.exp(-pred[:, 0].astype(np.float64)))
    return np.float32((p * dist).mean())


def _prep_in_maps(pred, target):
    bf16 = ml_dtypes.bfloat16
    mask = (target[:, 0] > 0).astype(np.float32)  # [B, H, W]
    in_maps = []
    for c in range(8):
        s, j = c // 2, c % 2
        r0 = j * HALF
        halo = np.zeros((HALO, W), np.float32)
        lo, hi = r0 - PAD, r0 + HALF + PAD
        slo, shi = max(lo, 0), min(hi, H)
        halo[slo - lo : shi - lo] = mask[s, slo:shi]
        # nbt[p, t, h] for column w = t*128+p -> pack as [128, 4*HALO]
        nbt_wh = (BIG * (1.0 - halo)).T  # [W, HALO]
        nbt = np.ascontiguousarray(
            nbt_wh.reshape(4, 128, HALO).transpose(1, 0, 2).reshape(128, 4 * HALO)
        ).astype(bf16)
        # pred[p, j2, w] for row r0 + j2*128 + p -> pack as [128, 2*W]
        ph = pred[s, 0, r0 : r0 + HALF, :].astype(np.float32)
        predh = np.ascontiguousarray(
            ph.reshape(2, 128, W).transpose(1, 0, 2).reshape(128, 2 * W)
        )
        in_maps.append({"nbt": nbt, "pred": predh})
    return in_maps


def kernel_with_results(pred, target, trace=False):
    """Returns (loss, BassKernelResults)."""
    global _compiled
    from concourse.bass_utils import run_bass_kernel_spmd

    if _compiled is None:
        _compiled = _build_bass()
    nc = _compiled

    in_maps = _prep_in_maps(pred, target)
    bkr = run_bass_kernel_spmd(nc, in_maps, core_ids=list(range(8)), trace=trace)

    has_fg = (target[:, 0] > 0).any(axis=(1, 2))  # [B]
    total = np.float64(0.0)
    cert = 0.0
    for c in range(8):
        s = c // 2
        if not has_fg[s]:
            continue
        out = bkr.results[c]["out"]  # [128, 4] f32
        total += np.float64(out[:, 0:2].sum(dtype=np.float64))
        cert = max(cert, float(out[:, 2].sum(dtype=np.float64)))

    if cert > 1e-3:
        # Windowed EDT not certified exact for this input; fall back.
        return _exact_loss_numpy(pred, target), bkr

    loss = np.array(total / (B * 1 * H * W), dtype=np.float32)
    return loss, bkr


def kernel(pred, target):
    loss, _ = kernel_with_results(pred, target)
    return loss


# revision 15
# speedup vs baseline: 1.2127x; 1.1709x over previous
"""Boundary loss kernel for Trainium2 (8 NeuronCores, SPMD).

loss = mean(sigmoid(pred) * EDT(target)) for pred/target [4,1,512,512].

Algorithm:
  The exact EDT dist2[y,x] = min over foreground (dy,dx) of dy^2+dx^2 is
  computed with a windowed separable min (window +-2): phase A does the
  vertical windowed min on a transposed [w, h] layout (shifts along the free
  dim), a TensorE transpose flips to [h, w], phase B does the horizontal
  windowed min. If every resulting dist2 <= K^2, the windowed result provably
  equals the exact EDT (a pixel with true distance <= K has its nearest
  foreground inside the window). The kernel also reduces
  sum(max(dist2 - K^2, 0)) as that exactness certificate; if it is nonzero
  (impossible for ~50%-dense random masks, where max distance is ~3) the host
  falls back to an exact numpy EDT — still correct, just slower on the host.

Sharding: core c handles sample c//2, row-half c%2 (256 rows + halo).

Performance notes:
  - scalar_tensor_tensor fuses shift+add+min in one VectorE op (1x-rate, so
    no alignment games are needed).
  - Host pre-packs inputs in the exact SBUF tile layout so DMAs are fully
    contiguous per partition.
  - Certificate reduction runs on GpSimd, sqrt/sigmoid on ScalarE, min-chains
    and the final fused multiply+sum on VectorE.
"""

import sys

sys.path.insert(0, "/opt/trn_rl_repo")

import numpy as np
import ml_dtypes

K = 3  # numpy-fallback window doc only; device window is +-2 (see CERT_T)
CERT_T = 8  # exactness certificate: dist2 <= 8 => |dy|,|dx| <= 2 => window hit
BIG = 16384.0
PAD = 4
B, H, W = 4, 512, 512
HALF = 256
HALO = HALF + 2 * PAD  # 264

_compiled = None


def _build_bass():
    import concourse.bacc as bacc
    import concourse.tile as tile
    from concourse import mybir

    # Bacc (not plain Bass): its compile pipeline runs register allocation
    # and generate_event_semaphores (splits multi-wait drains TRN2 codegen
    # rejects with "Too many sync wait commands").
    nc = bacc.Bacc(None)
    dt = mybir.dt
    Alu = mybir.AluOpType
    Act = mybir.ActivationFunctionType

    # Inputs are host-packed in SBUF layout: nbt[p, t, h] = BIG*(1-mask) at
    # column w = t*128+p, halo row h; pred[p, j, w] = logits at row j*128+p.
    nbt_d = nc.dram_tensor("nbt", [128, 4 * HALO], dt.bfloat16, kind="ExternalInput")
    pred_d = nc.dram_tensor("pred", [128, 2 * W], dt.float32, kind="ExternalInput")
    out_d = nc.dram_tensor("out", [128, 4], dt.float32, kind="ExternalOutput")
    ident_d = nc.inline_tensor(
        np.eye(128, dtype=ml_dtypes.bfloat16), name="ident_const"
    )

    with tile.TileContext(nc) as tc:
        with (
            tc.tile_pool(name="sb", bufs=1) as sb,
            tc.tile_pool(name="ps", bufs=2, space="PSUM") as ps,
        ):
            nbt = sb.tile([128, 4, HALO], dt.bfloat16)
            nc.sync.dma_start(out=nbt[:], in_=nbt_d[:].rearrange("p (t h) -> p t h", t=4))
            pred_sb = sb.tile([128, 2, W], dt.float32)
            nc.sync.dma_start(out=pred_sb[:], in_=pred_d[:].rearrange("p (j w) -> p j w", j=2))

            ident = sb.tile([128, 128], dt.bfloat16)
            nc.sync.dma_start(out=ident[:], in_=ident_d[:])

            # Sigmoid only needs pred: issue early so ScalarE does it while
            # VectorE runs phase A.
            sig = sb.tile([128, 2, W], dt.float32)
            nc.scalar.activation(out=sig[:], in_=pred_sb[:], func=Act.Sigmoid)

            # Phase A: vertical windowed min. Image row r0+h' is nbt index
            # PAD+h'; acc_v = min_dy nbt[PAD+h'+dy] + dy^2.
            acc_v = sb.tile([128, 4, HALF], dt.bfloat16)
            P = PAD
            stt = nc.vector.scalar_tensor_tensor
            # dy=+1 fused with dy=0 (first op, no init needed)
            stt(out=acc_v[:], in0=nbt[:, :, P + 1 : P + 1 + HALF], scalar=1.0,
                in1=nbt[:, :, P : P + HALF], op0=Alu.add, op1=Alu.min)
            for off, d2 in ((P - 1, 1.0), (P + 2, 4.0), (P - 2, 4.0)):
                stt(out=acc_v[:], in0=nbt[:, :, off : off + HALF], scalar=d2,
                    in1=acc_v[:], op0=Alu.add, op1=Alu.min)

            # Transpose [w, h] -> [h, w] via TensorE; land in padded m2vp
            # (data at [4, 516), pads = BIG so full-width phase-B ops read no
            # garbage at the edges).
            m2vp = sb.tile([128, 2, 520], dt.bfloat16)
            nc.gpsimd.memset(m2vp[:], BIG)
            for j in range(2):
                pt = ps.tile([128, 512], dt.bfloat16)
                for t in range(4):
                    nc.tensor.transpose(
                        out=pt[:, t * 128 : (t + 1) * 128],
                        in_=acc_v[:, t, j * 128 : (j + 1) * 128],
                        identity=ident[:],
                    )
                nc.scalar.copy(out=m2vp[:, j, 4:516], in_=pt[:])

            # Phase B: horizontal windowed min, full-width ops (data base 4).
            acc_h = sb.tile([128, 2, W], dt.bfloat16)
            stt(out=acc_h[:], in0=m2vp[:, :, 5:517], scalar=1.0,
                in1=m2vp[:, :, 4:516], op0=Alu.add, op1=Alu.min)  # dx=+1, 0
            for off, d2 in ((3, 1.0), (6, 4.0), (2, 4.0)):
                stt(out=acc_h[:], in0=m2vp[:, :, off : off + W], scalar=d2,
                    in1=acc_h[:], op0=Alu.add, op1=Alu.min)

            out_sb = sb.tile([128, 4], dt.float32)
            nc.gpsimd.memset(out_sb[:], 0.0)

            # Exactness certificate (walrus rejects tensor_scalar on GpSimd).
            cert_junk = sb.tile([128, 2, W], dt.bfloat16)
            nc.vector.tensor_scalar(
                out=cert_junk[:], in0=acc_h[:],
                scalar1=-float(CERT_T), scalar2=0.0,
                op0=Alu.add, op1=Alu.max,
                accum_out=out_sb[:, 2:3],
            )

            # Tail, split per row-half so stt(j0) overlaps sqrt(j1).
            dist = sb.tile([128, 2, W], dt.float32)
            prod_junk = sb.tile([128, 2, W], dt.float32)
            for j in range(2):
                nc.scalar.activation(out=dist[:, j, :], in_=acc_h[:, j, :], func=Act.Sqrt)
                nc.vector.scalar_tensor_tensor(
                    out=prod_junk[:, j, :], in0=sig[:, j, :], scalar=1.0,
                    in1=dist[:, j, :], op0=Alu.mult, op1=Alu.mult,
                    accum_out=out_sb[:, j : j + 1],
                )

            nc.sync.dma_start(out=out_d[:], in_=out_sb[:])

    nc.finalize()
    return nc


def _exact_loss_numpy(pred, target):
    """Exact fallback, matching reference.py semantics."""
    mask = target[:, 0].astype(np.float32)
    b, h, w = mask.shape
    big = np.float32(h + w)
    rows = np.arange(h, dtype=np.float32)[None, :, None]
    fg = mask > 0
    last = np.maximum.accumulate(np.where(fg, rows, -big), axis=1)
    nxt = np.minimum.accumulate(np.where(fg, rows, 3 * big)[:, ::-1], axis=1)[:, ::-1]
    g = np.minimum(np.minimum(rows - last, nxt - rows), big)
    g2 = (g * g).astype(np.float32)
    cols = np.arange(w, dtype=np.float32)
    diff2 = (cols[:, None] - cols[None, :]) ** 2
    dist = np.empty((b, h, w), np.float32)
    for bi in range(b):
        for r0 in range(0, h, 64):
            blk = g2[bi, r0 : r0 + 64]
            dist[bi, r0 : r0 + 64] = np.sqrt(
                (diff2[None, :, :] + blk[:, None, :]).min(-1)
            )
    has_fg = fg.any(axis=(1, 2))
    dist = np.where(has_fg[:, None, None], dist, 0.0)
    p = 1.0 / (1.0 + np.exp(-pred[:, 0].astype(np.float64)))
    return np.float32((p * dist).mean())


def _prep_in_maps(pred, target):
    bf16 = ml_dtypes.bfloat16
    mask = (target[:, 0] > 0).astype(np.float32)  # [B, H, W]
    in_maps = []
    for c in range(8):
        s, j = c // 2, c % 2
        r0 = j * HALF
        halo = np.zeros((HALO, W), np.float32)
        lo, hi = r0 - PAD, r0 + HALF + PAD
        slo, shi = max(lo, 0), min(hi, H)
        halo[slo - lo : shi - lo] = mask[s, slo:shi]
        # nbt[p, t, h] for column w = t*128+p -> pack as [128, 4*HALO]
        nbt_wh = (BIG * (1.0 - halo)).T  # [W, HALO]
        nbt = np.ascontiguousarray(
            nbt_wh.reshape(4, 128, HALO).transpose(1, 0, 2).reshape(128, 4 * HALO)
        ).astype(bf16)
        # pred[p, j2, w] for row r0 + j2*128 + p -> pack as [128, 2*W]
        ph = pred[s, 0, r0 : r0 + HALF, :].astype(np.float32)
        predh = np.ascontiguousarray(
            ph.reshape(2, 128, W).transpose(1, 0, 2).reshape(128, 2 * W)
        )
        in_maps.append({"nbt": nbt, "pred": predh})
    return in_maps


def kernel_with_results(pred, target, trace=False):
    """Returns (loss, BassKernelResults)."""
    global _compiled
    from concourse.bass_utils import run_bass_kernel_spmd

    if _compiled is None:
        _compiled = _build_bass()
    nc = _compiled

    in_maps = _prep_in_maps(pred, target)
    bkr = run_bass_kernel_spmd(nc, in_maps, core_ids=list(range(8)), trace=trace)

    has_fg = (target[:, 0] > 0).any(axis=(1, 2))  # [B]
    total = np.float64(0.0)
    cert = 0.0
    for c in range(8):
        s = c // 2
        if not has_fg[s]:
            continue
        out = bkr.results[c]["out"]  # [128, 4] f32
        total += np.float64(out[:, 0:2].sum(dtype=np.float64))
        cert = max(cert, float(out[:, 2].sum(dtype=np.float64)))

    if cert > 1e-3:
        # Windowed EDT not certified exact for this input; fall back.
        return _exact_loss_numpy(pred, target), bkr

    loss = np.array(total / (B * 1 * H * W), dtype=np.float32)
    return loss, bkr


def kernel(pred, target):
    loss, _ = kernel_with_results(pred, target)
    return loss
